# revision 39
# baseline (speedup 1.0000x reference)
"""Trainium2 Bass kernel for nn_MultiHeadAttention_4810363372776 (linear attention).

Sharding: data-parallel over batch (4) x tensor-parallel over head groups (2).
Core i handles batch i//2, heads [8*(i%2), 8*(i%2)+8). Each core computes its
partial output projection; the host sums the two head-group partials per batch.

q/k/v are pre-transposed to [D, S] on the host (fp8 for q/k, bf16 for v), so
the device does no input transposes. q/k projections run as fp8 DoubleRow
matmuls; the v path (v-proj, kv, ctx, out-proj) stays bf16 — any fp8 stage
there costs ~3.6% l2 error on the output, which exceeds the 2e-2 gate.
"""

import functools
import numpy as np

B, S, D, H = 4, 4096, 1024, 16
DK = D // H          # 64
OG = D // 2          # 512 per-core head-group width (8 heads)
NCORES = 8
SCALE = 1.0 / 8.0    # 1/sqrt(DK)
NT = S // 128        # 32 s-tiles
MACRO = 1024
NM = S // MACRO      # 4 macros
NU = MACRO // 128    # 8 s-tiles per macro


@functools.lru_cache(maxsize=2)
def _build(kv_bias=False):
    import concourse.bass as bass  # noqa: F401
    from concourse import bacc
    import concourse.mybir as mybir
    import concourse.tile as tile
    from concourse.masks import make_identity
    from contextlib import ExitStack

    f32 = mybir.dt.float32
    bf16 = mybir.dt.bfloat16
    fp8 = mybir.dt.float8e4
    DR = mybir.MatmulPerfMode.DoubleRow
    EXP = mybir.ActivationFunctionType.Exp
    COPY = mybir.ActivationFunctionType.Copy
    AXX = mybir.AxisListType.X
    ADD = mybir.AluOpType.add

    nc = bacc.Bacc()

    # xk/xv are tile-blocked [p, s-tile, d-chunk, s-in-tile] so each macro
    # load is one contiguous line per partition (128 fat descriptors).
    xqt = nc.declare_dram_parameter("xqt", [D, S], fp8, isOutput=False)
    xkt = nc.declare_dram_parameter("xkt", [128, NT, 8, 128], fp8, isOutput=False)
    xvt = nc.declare_dram_parameter("xvt", [128, NT, 8, 128], bf16, isOutput=False)
    wqt = nc.declare_dram_parameter("wqt", [128, 8, OG], fp8, isOutput=False)
    wkt = nc.declare_dram_parameter("wkt", [128, 8, OG], fp8, isOutput=False)
    wvt = nc.declare_dram_parameter("wvt", [128, 8, OG], bf16, isOutput=False)
    wot = nc.declare_dram_parameter("wot", [128, 4, D], bf16, isOutput=False)
    bqsp = nc.declare_dram_parameter("bqs", [128, 4], f32, isOutput=False)
    bkp = nc.declare_dram_parameter("bk", [1, OG], f32, isOutput=False)
    bvp = nc.declare_dram_parameter("bv", [1, OG], f32, isOutput=False)
    maskp = nc.declare_dram_parameter("maskf", [128, NT], f32, isOutput=False)
    out = nc.declare_dram_parameter("out", [S, D], bf16, isOutput=True)

    with tile.TileContext(nc) as tc:
        with ExitStack() as ctx:
            singles = ctx.enter_context(tc.tile_pool(name="singles", bufs=1))

            ident = singles.tile([128, 128], bf16)
            make_identity(nc, ident)

            # all phase-1 loads go on the gpsimd queue in exact need-order
            # (single-queue FIFO = strict priority at full bandwidth)
            wk_sb = singles.tile([128, 8, OG], fp8, tag="wk")
            mask_sb = singles.tile([128, NT], f32, tag="mask")
            wv_sb = singles.tile([128, 8, OG], bf16, tag="wv")
            bqs_sb = singles.tile([128, 4], f32, tag="bqs")
            wq_sb = singles.tile([128, 8, OG], fp8, tag="wq")
            wo_sb = singles.tile([128, 4, D], bf16, tag="wo")
            nc.gpsimd.dma_start(out=wk_sb, in_=wkt[:, :, :])
            if kv_bias:
                bk_bc = singles.tile([128, OG], f32, tag="bk_bc")
                nc.gpsimd.dma_start(out=bk_bc, in_=bkp[:, :].partition_broadcast(128))
                bv_bc = singles.tile([128, OG], f32, tag="bv_bc")
                nc.gpsimd.dma_start(out=bv_bc, in_=bvp[:, :].partition_broadcast(128))
            # full q^T resident (fp8, 32KB/partition); q-proj runs in phase 2
            xq_sb = singles.tile([128, 8, S], fp8, tag="xq")
            # block-diag [kv | ksum] per head pair; memset early (off critical path)
            kvbd = [singles.tile([128, 130], bf16, tag=f"kvbd{p}", name=f"kvbd{p}") for p in range(4)]
            for p in range(4):
                nc.vector.memset(kvbd[p], 0.0)

            # ---------------- phase 1 ----------------
            with ExitStack() as p1:
                pacc_pool = p1.enter_context(tc.tile_pool(name="pacc", bufs=1, space="PSUM"))
                # two chains per bank; bank-wide has_written clear happens once (st==0, even pair)
                kvps = [pacc_pool.tile([128, 2, 129], f32, tag=f"kvacc{i}", name=f"kvacc{i}") for i in range(2)]
                xin_pool = p1.enter_context(tc.tile_pool(name="xin", bufs=2))
                kvf_pool = p1.enter_context(tc.tile_pool(name="kvf", bufs=3))
                pkv_pool = p1.enter_context(tc.tile_pool(name="pkv", bufs=4, space="PSUM"))

                # PE warmup: get HAM to K=8/8 before the first real matmul
                # lands (accumulation group, result never read)
                pwarm_pool = p1.enter_context(tc.tile_pool(name="pwarm", bufs=1, space="PSUM"))
                wps = pwarm_pool.tile([128, 128], f32, tag="warm")
                for i in range(38):
                    nc.tensor.matmul(wps, ident, ident, start=(i == 0), stop=(i == 37))

                pending = None  # (kf, vf, st) deferred kv accumulation

                def flush_kv(pending):
                    kf, vf, pst = pending
                    for p in range(4):
                        nc.tensor.matmul(
                            kvps[p // 2][:, p % 2, 0:129],
                            kf[:, 2 * p:2 * p + 2, :],
                            vf[:, p, 0:129],
                            start=(pst == 0 and p % 2 == 0),
                            stop=(pst == NT - 1),
                            skip_group_check=True,
                        )

                # Variable-width macros: small ones first so compute starts
                # as soon as the first slices land (deps are tile-granular).
                MACROS = [256, 256, 512, 1024, 1024, 1024]
                assert sum(MACROS) == S
                s_base = 0
                for a, W in enumerate(MACROS):
                    NUw = W // 128
                    st0 = s_base // 128
                    # k first (first compute), q last (needed only at q-chunk)
                    xk_sb = xin_pool.tile([128, 8, 8, 128], fp8, tag="xk")
                    nc.gpsimd.dma_start(out=xk_sb[:, 0:NUw], in_=xkt[:, st0:st0 + NUw])
                    if a == 0:
                        nc.gpsimd.dma_start(out=wv_sb, in_=wvt[:, :, :])
                    xv_sb = xin_pool.tile([128, 8, 8, 128], bf16, tag="xv")
                    nc.gpsimd.dma_start(out=xv_sb[:, 0:NUw], in_=xvt[:, st0:st0 + NUw])
                    if a == 0:
                        nc.gpsimd.dma_start(out=mask_sb, in_=maskp[:, :])
                    if a == 1:
                        # phase-2-only loads, after the phase-1 head prefix
                        nc.gpsimd.dma_start(out=bqs_sb, in_=bqsp[:, :])
                        nc.gpsimd.dma_start(out=wq_sb, in_=wqt[:, :, :])
                        nc.gpsimd.dma_start(out=xq_sb, in_=xqt[:, :].rearrange("(t p) s -> p t s", p=128))

                    for u in range(NUw):
                        st = st0 + u
                        sl = slice(u * 128, (u + 1) * 128)

                        # k projection (fp8 DoubleRow over 1024 contract)
                        pk = pkv_pool.tile([128, OG], f32, tag="pkv")
                        for t2 in range(4):
                            nc.tensor.matmul(pk, xk_sb[:, u, 2 * t2:2 * t2 + 2, :],
                                             wk_sb[:, 2 * t2:2 * t2 + 2, :],
                                             start=(t2 == 0), stop=(t2 == 3), perf_mode=DR)
                        if kv_bias:
                            nc.vector.tensor_add(pk, pk, bk_bc)
                        ek = kvf_pool.tile([128, OG], bf16, tag="ek")
                        nc.scalar.activation(ek, pk, EXP, scale=SCALE)
                        rows = kvf_pool.tile([128, 8], f32, tag="rows")
                        nc.vector.tensor_reduce(rows, ek.rearrange("p (h e) -> p h e", h=8), axis=AXX, op=ADD)
                        nc.vector.reciprocal(rows, rows)
                        nc.vector.tensor_scalar_mul(rows, rows, mask_sb[:, st:st + 1])
                        kf = kvf_pool.tile([128, 8, DK], bf16, tag="kf")
                        nc.vector.tensor_mul(
                            kf,
                            ek.rearrange("p (h e) -> p h e", h=8),
                            rows[:, :, None].to_broadcast([128, 8, DK]),
                        )

                        # v projection (bf16)
                        pv = pkv_pool.tile([128, OG], f32, tag="pkv")
                        for t in range(8):
                            nc.tensor.matmul(pv, xv_sb[:, u, t, :], wv_sb[:, t, :],
                                             start=(t == 0), stop=(t == 7))
                        if kv_bias:
                            nc.vector.tensor_add(pv, pv, bv_bc)
                        vf = kvf_pool.tile([128, 4, 130], bf16, tag="vf")
                        nc.scalar.activation(vf[:, :, 0:128], pv.rearrange("p (j s) -> p j s", j=4), COPY, scale=mask_sb[:, st:st + 1])
                        nc.vector.memset(vf[:, :, 128:129], 1.0)

                        # deferred kv accumulation for the previous s-tile
                        if pending is not None:
                            flush_kv(pending)
                        pending = (kf, vf, st)

                    if a == 0:
                        # wo is phase-2-only; load it once the urgent head
                        # transfers are queued
                        nc.sync.dma_start(out=wo_sb, in_=wot[:, :, :])
                    s_base += W

                if pending is not None:
                    flush_kv(pending)

                # build block-diag [kv | ksum] tiles (bf16); split DVE/ACT
                for p in range(4):
                    ps = kvps[p // 2][:, p % 2]
                    nc.vector.tensor_copy(kvbd[p][0:64, 0:64], ps[0:64, 0:64])
                    nc.vector.tensor_copy(kvbd[p][0:64, 64:65], ps[0:64, 128:129])
                    nc.scalar.copy(out=kvbd[p][64:128, 65:129], in_=ps[64:128, 64:128])
                    nc.scalar.copy(out=kvbd[p][64:128, 129:130], in_=ps[64:128, 128:129])

            # ---------------- phase 2 ----------------
            # stages per s-tile: num -> (DVE) ctx -> (PE) ctxT -> (ACT) evac -> (PE) out-proj
            # software-pipelined: ctxT lags one tile, out-proj lags two.
            with ExitStack() as p2s:
                p2 = p2s.enter_context(tc.tile_pool(name="p2", bufs=3))
                et_pool = p2s.enter_context(tc.tile_pool(name="et", bufs=3))
                pnum_pool = p2s.enter_context(tc.tile_pool(name="pnum", bufs=1, space="PSUM"))
                pct_pool = p2s.enter_context(tc.tile_pool(name="pct", bufs=2, space="PSUM"))
                po_pool = p2s.enter_context(tc.tile_pool(name="po", bufs=2, space="PSUM"))
                pq_pool = p2s.enter_context(tc.tile_pool(name="pq", bufs=2, space="PSUM"))

                ctx_q = {}   # st -> ctx tile
                ctxT_q = {}  # st -> ctxT tile
                et_q = {}    # chunk -> ET chunk tile [128, 4, 512]

                def stage_qproj(c):
                    # exp(q_hat*scale + bq) for s-chunk c, transposed [o, s]
                    et = et_pool.tile([128, 4, 512], bf16, tag="et", name="et")
                    for ob in range(4):
                        pq = pq_pool.tile([128, 512], f32, tag="pq", name="pq")
                        for t2 in range(4):
                            nc.tensor.matmul(pq, wq_sb[:, 2 * t2:2 * t2 + 2, ob * 128:(ob + 1) * 128],
                                             xq_sb[:, 2 * t2:2 * t2 + 2, c * 512:(c + 1) * 512],
                                             start=(t2 == 0), stop=(t2 == 3), perf_mode=DR)
                        nc.scalar.activation(et[:, ob, :], pq, EXP, bias=bqs_sb[:, ob:ob + 1], scale=SCALE)
                    et_q[c] = et

                def stage_num(st):
                    s0 = (st % 4) * 128
                    et = et_q[st // 4]
                    pnums = [pnum_pool.tile([128, 2, 130], f32, tag=f"pnum{i}", name=f"pnum{i}") for i in range(2)]
                    for p in range(4):
                        nc.tensor.matmul(pnums[p // 2][:, p % 2, :], et[:, p, s0:s0 + 128], kvbd[p], start=True, stop=True)
                    if st % 4 == 3:
                        del et_q[st // 4]
                    ctxs = p2.tile([128, OG], bf16, tag="ctx", name="ctxs")
                    for i in range(2):
                        pn4 = pnums[i].rearrange("p j (two c) -> p (j two) c", two=2)  # [128, 4, 65]
                        r4 = p2.tile([128, 4, 1], f32, tag="r", name="r4")
                        nc.vector.reciprocal(r4, pn4[:, :, 64:65])
                        ctx4 = ctxs[:, i * 256:(i + 1) * 256].rearrange("p (j c) -> p j c", c=64)
                        nc.vector.tensor_mul(ctx4, pn4[:, :, 0:64], r4.to_broadcast([128, 4, 64]))
                    ctx_q[st] = ctxs

                def stage_ctxT(st):
                    ctxs = ctx_q.pop(st)
                    pct = pct_pool.tile([128, 512], bf16, tag="pct", name="pct")
                    for eb in range(4):
                        nc.tensor.transpose(pct[:, eb * 128:(eb + 1) * 128], ctxs[:, eb * 128:(eb + 1) * 128], ident)
                    ctxT = p2.tile([128, 4, 128], bf16, tag="ctxT", name="ctxT")
                    nc.scalar.copy(out=ctxT, in_=pct.rearrange("p (j s) -> p j s", j=4))
                    ctxT_q[st] = ctxT

                def stage_oproj(st):
                    s0 = st * 128
                    ctxT = ctxT_q.pop(st)
                    outsb = p2.tile([128, D], bf16, tag="outsb", name="outsb")
                    for half in range(2):
                        po = po_pool.tile([128, 512], f32, tag="po", name="po")
                        for eb in range(4):
                            nc.tensor.matmul(po, ctxT[:, eb, :],
                                             wo_sb[:, eb, half * 512:(half + 1) * 512],
                                             start=(eb == 0), stop=(eb == 3))
                        if half == 0:
                            nc.scalar.copy(out=outsb[:, 0:512], in_=po)
                        else:
                            nc.vector.tensor_copy(outsb[:, 512:1024], po)
                    # gpsimd queue is idle in phase 2; sync is clogged with sems
                    nc.gpsimd.dma_start(out=out[s0:s0 + 128, :], in_=outsb)


                stage_qproj(0)
                for st in range(NT):
                    stage_num(st)
                    if st % 4 == 2 and st // 4 + 1 < NT // 4:
                        stage_qproj(st // 4 + 1)
                    if st >= 1:
                        stage_ctxT(st - 1)
                    if st >= 2:
                        stage_oproj(st - 2)
                stage_ctxT(NT - 1)
                stage_oproj(NT - 2)
                stage_oproj(NT - 1)

    nc.compile()
    return nc


_LAST_RESULT = None


def kernel(q, k, v, mask, Wq, bq, Wk, bk, Wv, bv, Wo, bo):
    global _LAST_RESULT
    import ml_dtypes
    from concourse.bass_utils import run_bass_kernel_spmd

    q = np.asarray(q, np.float32)
    k = np.asarray(k, np.float32)
    v = np.asarray(v, np.float32)
    mask = np.asarray(mask)
    Wq = np.asarray(Wq, np.float32)
    Wk = np.asarray(Wk, np.float32)
    Wv = np.asarray(Wv, np.float32)
    Wo = np.asarray(Wo, np.float32)
    bq = np.asarray(bq, np.float32)
    bk = np.asarray(bk, np.float32)
    bv = np.asarray(bv, np.float32)
    bo = np.asarray(bo, np.float32)

    nc = _build(bool(np.any(bk) or np.any(bv)))

    bf = ml_dtypes.bfloat16
    f8 = ml_dtypes.float8_e4m3
    in_maps = []
    xts = {}
    def wblocked(wt, t):
        # [T*128, O] -> [128 p, T d-chunks, O] (contiguous per partition)
        return np.ascontiguousarray(wt.reshape(t, 128, -1).transpose(1, 0, 2))

    def blocked(xt):
        # [D, S] -> [128 p, NT s-tiles, 8 d-chunks, 128 s] (contiguous per
        # partition per s-tile range)
        return np.ascontiguousarray(xt.reshape(8, 128, NT, 128).transpose(1, 2, 0, 3))

    for b in range(B):
        xts[b] = (
            np.ascontiguousarray(q[b].T).astype(f8),
            blocked(np.ascontiguousarray(k[b].T).astype(f8)),
            blocked(np.ascontiguousarray(v[b].T).astype(bf)),
        )
    for core in range(NCORES):
        b, g = core // 2, core % 2
        sl = slice(g * OG, (g + 1) * OG)
        maskf = mask[b, 0, 0, :].astype(np.float32).reshape(NT, 128).T.copy()
        xq8, xk8, xv8 = xts[b]
        in_maps.append({
            "xqt": xq8,
            "xkt": xk8,
            "xvt": xv8,
            "wqt": wblocked(np.ascontiguousarray(Wq[sl, :].T).astype(f8), 8),
            "wkt": wblocked(np.ascontiguousarray(Wk[sl, :].T).astype(f8), 8),
            "wvt": wblocked(np.ascontiguousarray(Wv[sl, :].T).astype(bf), 8),
            "wot": wblocked(np.ascontiguousarray(Wo[:, sl].T).astype(bf), 4),
            "bqs": np.ascontiguousarray((bq[sl] * SCALE).reshape(4, 128).T),
            "bk": bk[sl].reshape(1, OG).copy(),
            "bv": bv[sl].reshape(1, OG).copy(),
            "maskf": maskf,
        })

    res = run_bass_kernel_spmd(nc, in_maps, list(range(NCORES)))
    _LAST_RESULT = res

    outp = np.empty((B, S, D), np.float32)
    for b in range(B):
        outp[b] = (res.results[2 * b]["out"].astype(np.float32)
                   + res.results[2 * b + 1]["out"].astype(np.float32)
                   + bo[None, :])
    return outp


# revision 40
# speedup vs baseline: 1.1805x; 1.1805x over previous
"""Trainium2 Bass kernel for nn_MultiHeadAttention_4810363372776 (linear attention).

Sharding: data-parallel over batch (4) x tensor-parallel over head groups (2).
Core i handles batch i//2, heads [8*(i%2), 8*(i%2)+8). Each core computes its
partial output projection; the host sums the two head-group partials per batch.

q/k/v are pre-transposed to [D, S] on the host (fp8 for q/k, bf16 for v), so
the device does no input transposes. q/k projections run as fp8 DoubleRow
matmuls; the v path (v-proj, kv, ctx, out-proj) stays bf16 — any fp8 stage
there costs ~3.6% l2 error on the output, which exceeds the 2e-2 gate.
"""

import functools
import numpy as np

B, S, D, H = 4, 4096, 1024, 16
DK = D // H          # 64
OG = D // 2          # 512 per-core head-group width (8 heads)
NCORES = 8
SCALE = 1.0 / 8.0    # 1/sqrt(DK)
NT = S // 128        # 32 s-tiles
MACRO = 1024
NM = S // MACRO      # 4 macros
NU = MACRO // 128    # 8 s-tiles per macro


@functools.lru_cache(maxsize=2)
def _build(kv_bias=False):
    import concourse.bass as bass  # noqa: F401
    from concourse import bacc
    import concourse.mybir as mybir
    import concourse.tile as tile
    from concourse.masks import make_identity
    from contextlib import ExitStack

    f32 = mybir.dt.float32
    bf16 = mybir.dt.bfloat16
    fp8 = mybir.dt.float8e4
    DR = mybir.MatmulPerfMode.DoubleRow
    EXP = mybir.ActivationFunctionType.Exp
    COPY = mybir.ActivationFunctionType.Copy
    AXX = mybir.AxisListType.X
    ADD = mybir.AluOpType.add

    nc = bacc.Bacc()

    # xk/xv are tile-blocked [p, s-tile, d-chunk, s-in-tile] so each macro
    # load is one contiguous line per partition (128 fat descriptors).
    xqt = nc.declare_dram_parameter("xqt", [D, S], fp8, isOutput=False)
    xkt = nc.declare_dram_parameter("xkt", [128, NT, 8, 128], fp8, isOutput=False)
    xvt = nc.declare_dram_parameter("xvt", [128, NT, 8, 128], bf16, isOutput=False)
    wqt = nc.declare_dram_parameter("wqt", [128, 8, OG], fp8, isOutput=False)
    wkt = nc.declare_dram_parameter("wkt", [128, 8, OG], fp8, isOutput=False)
    wvt = nc.declare_dram_parameter("wvt", [128, 8, OG], bf16, isOutput=False)
    wot = nc.declare_dram_parameter("wot", [128, 4, D], bf16, isOutput=False)
    bqsp = nc.declare_dram_parameter("bqs", [128, 4], f32, isOutput=False)
    bkp = nc.declare_dram_parameter("bk", [1, OG], f32, isOutput=False)
    bvp = nc.declare_dram_parameter("bv", [1, OG], f32, isOutput=False)
    maskp = nc.declare_dram_parameter("maskf", [128, NT], f32, isOutput=False)
    out = nc.declare_dram_parameter("out", [S, D], bf16, isOutput=True)

    with tile.TileContext(nc) as tc:
        with ExitStack() as ctx:
            singles = ctx.enter_context(tc.tile_pool(name="singles", bufs=1))

            ident = singles.tile([128, 128], bf16)
            make_identity(nc, ident)

            # all phase-1 loads go on the gpsimd queue in exact need-order
            # (single-queue FIFO = strict priority at full bandwidth)
            wk_sb = singles.tile([128, 8, OG], fp8, tag="wk")
            mask_sb = singles.tile([128, NT], f32, tag="mask")
            wv_sb = singles.tile([128, 8, OG], bf16, tag="wv")
            bqs_sb = singles.tile([128, 4], f32, tag="bqs")
            wq_sb = singles.tile([128, 8, OG], fp8, tag="wq")
            wo_sb = singles.tile([128, 4, D], bf16, tag="wo")
            nc.gpsimd.dma_start(out=wk_sb, in_=wkt[:, :, :])
            if kv_bias:
                bk_bc = singles.tile([128, OG], f32, tag="bk_bc")
                nc.gpsimd.dma_start(out=bk_bc, in_=bkp[:, :].partition_broadcast(128))
                bv_bc = singles.tile([128, OG], f32, tag="bv_bc")
                nc.gpsimd.dma_start(out=bv_bc, in_=bvp[:, :].partition_broadcast(128))
            # exp(q_hat * scale), stored [o (4 blocks of 128 = head pairs), s]
            ET = singles.tile([128, 4, S], bf16, tag="ET")
            # block-diag [kv | ksum] per head pair; memset early (off critical path)
            kvbd = [singles.tile([128, 130], bf16, tag=f"kvbd{p}", name=f"kvbd{p}") for p in range(4)]
            for p in range(4):
                nc.vector.memset(kvbd[p], 0.0)

            # ---------------- phase 1 ----------------
            with ExitStack() as p1:
                pacc_pool = p1.enter_context(tc.tile_pool(name="pacc", bufs=1, space="PSUM"))
                # two chains per bank; bank-wide has_written clear happens once (st==0, even pair)
                kvps = [pacc_pool.tile([128, 2, 129], f32, tag=f"kvacc{i}", name=f"kvacc{i}") for i in range(2)]
                xin_pool = p1.enter_context(tc.tile_pool(name="xin", bufs=2))
                kvf_pool = p1.enter_context(tc.tile_pool(name="kvf", bufs=3))
                pkv_pool = p1.enter_context(tc.tile_pool(name="pkv", bufs=4, space="PSUM"))

                # PE warmup: get HAM to K=8/8 before the first real matmul
                # lands (accumulation group, result never read)
                pwarm_pool = p1.enter_context(tc.tile_pool(name="pwarm", bufs=1, space="PSUM"))
                wps = pwarm_pool.tile([128, 128], f32, tag="warm")
                for i in range(38):
                    nc.tensor.matmul(wps, ident, ident, start=(i == 0), stop=(i == 37))

                pending = None  # (kf, vf, st) deferred kv accumulation

                def flush_kv(pending):
                    kf, vf, pst = pending
                    for p in range(4):
                        nc.tensor.matmul(
                            kvps[p // 2][:, p % 2, 0:129],
                            kf[:, 2 * p:2 * p + 2, :],
                            vf[:, p, 0:129],
                            start=(pst == 0 and p % 2 == 0),
                            stop=(pst == NT - 1),
                            skip_group_check=True,
                        )

                # Variable-width macros: small ones first so compute starts
                # as soon as the first slices land (deps are tile-granular).
                MACROS = [256, 256, 512, 1024, 1024, 1024]
                assert sum(MACROS) == S
                s_base = 0
                for a, W in enumerate(MACROS):
                    NUw = W // 128
                    st0 = s_base // 128
                    # k first (first compute), q last (needed only at q-chunk)
                    xk_sb = xin_pool.tile([128, 8, 8, 128], fp8, tag="xk")
                    nc.gpsimd.dma_start(out=xk_sb[:, 0:NUw], in_=xkt[:, st0:st0 + NUw])
                    if a == 0:
                        nc.gpsimd.dma_start(out=wv_sb, in_=wvt[:, :, :])
                    xv_sb = xin_pool.tile([128, 8, 8, 128], bf16, tag="xv")
                    nc.gpsimd.dma_start(out=xv_sb[:, 0:NUw], in_=xvt[:, st0:st0 + NUw])
                    if a == 0:
                        nc.gpsimd.dma_start(out=mask_sb, in_=maskp[:, :])
                        nc.gpsimd.dma_start(out=bqs_sb, in_=bqsp[:, :])
                        nc.gpsimd.dma_start(out=wq_sb, in_=wqt[:, :, :])
                    xq_sb = xin_pool.tile([128, 8, MACRO], fp8, tag="xq")
                    nc.gpsimd.dma_start(out=xq_sb[:, :, 0:W], in_=xqt[:, s_base:s_base + W].rearrange("(t p) s -> p t s", p=128))

                    for u in range(NUw):
                        st = st0 + u
                        sl = slice(u * 128, (u + 1) * 128)

                        # k projection (fp8 DoubleRow over 1024 contract)
                        pk = pkv_pool.tile([128, OG], f32, tag="pkv")
                        for t2 in range(4):
                            nc.tensor.matmul(pk, xk_sb[:, u, 2 * t2:2 * t2 + 2, :],
                                             wk_sb[:, 2 * t2:2 * t2 + 2, :],
                                             start=(t2 == 0), stop=(t2 == 3), perf_mode=DR)
                        if kv_bias:
                            nc.vector.tensor_add(pk, pk, bk_bc)
                        ek = kvf_pool.tile([128, OG], bf16, tag="ek")
                        nc.scalar.activation(ek, pk, EXP, scale=SCALE)
                        rows = kvf_pool.tile([128, 8], f32, tag="rows")
                        nc.vector.tensor_reduce(rows, ek.rearrange("p (h e) -> p h e", h=8), axis=AXX, op=ADD)
                        nc.vector.reciprocal(rows, rows)
                        nc.vector.tensor_scalar_mul(rows, rows, mask_sb[:, st:st + 1])
                        kf = kvf_pool.tile([128, 8, DK], bf16, tag="kf")
                        nc.vector.tensor_mul(
                            kf,
                            ek.rearrange("p (h e) -> p h e", h=8),
                            rows[:, :, None].to_broadcast([128, 8, DK]),
                        )

                        # v projection (bf16)
                        pv = pkv_pool.tile([128, OG], f32, tag="pkv")
                        for t in range(8):
                            nc.tensor.matmul(pv, xv_sb[:, u, t, :], wv_sb[:, t, :],
                                             start=(t == 0), stop=(t == 7))
                        if kv_bias:
                            nc.vector.tensor_add(pv, pv, bv_bc)
                        vf = kvf_pool.tile([128, 4, 130], bf16, tag="vf")
                        nc.scalar.activation(vf[:, :, 0:128], pv.rearrange("p (j s) -> p j s", j=4), COPY, scale=mask_sb[:, st:st + 1])
                        nc.vector.memset(vf[:, :, 128:129], 1.0)

                        # deferred kv accumulation for the previous s-tile
                        if pending is not None:
                            flush_kv(pending)
                        pending = (kf, vf, st)

                        # q projection per chunk (<=512 wide), output [o, s]
                        QW = min(512, W)
                        if (u + 1) * 128 % QW == 0:
                            c = (u * 128) // QW
                            soff = s_base + c * QW
                            last_chunk = (a == len(MACROS) - 1) and (u == NUw - 1)
                            for ob in range(4):
                                pq = pkv_pool.tile([128, 512], f32, tag="pkv")
                                for t2 in range(4):
                                    nc.tensor.matmul(pq[:, 0:QW], wq_sb[:, 2 * t2:2 * t2 + 2, ob * 128:(ob + 1) * 128],
                                                     xq_sb[:, 2 * t2:2 * t2 + 2, c * QW:(c + 1) * QW],
                                                     start=(t2 == 0), stop=(t2 == 3), perf_mode=DR)
                                nc.scalar.activation(ET[:, ob, soff:soff + QW], pq[:, 0:QW], EXP, bias=bqs_sb[:, ob:ob + 1], scale=SCALE)
                                if last_chunk and ob == 0:
                                    # flush the final tile's kv now so the
                                    # kvbd build overlaps the rest of q-proj
                                    flush_kv(pending)
                                    pending = None

                    if a == 0:
                        # wo is phase-2-only; load it once the urgent head
                        # transfers are queued
                        nc.sync.dma_start(out=wo_sb, in_=wot[:, :, :])
                    s_base += W

                if pending is not None:
                    flush_kv(pending)

                # build block-diag [kv | ksum] tiles (bf16); split DVE/ACT
                for p in range(4):
                    ps = kvps[p // 2][:, p % 2]
                    nc.vector.tensor_copy(kvbd[p][0:64, 0:64], ps[0:64, 0:64])
                    nc.vector.tensor_copy(kvbd[p][0:64, 64:65], ps[0:64, 128:129])
                    nc.scalar.copy(out=kvbd[p][64:128, 65:129], in_=ps[64:128, 64:128])
                    nc.scalar.copy(out=kvbd[p][64:128, 129:130], in_=ps[64:128, 128:129])

            # ---------------- phase 2 ----------------
            # stages per s-tile: num -> (DVE) ctx -> (PE) ctxT -> (ACT) evac -> (PE) out-proj
            # software-pipelined: ctxT lags one tile, out-proj lags two.
            with ExitStack() as p2s:
                p2 = p2s.enter_context(tc.tile_pool(name="p2", bufs=3))
                pnum_pool = p2s.enter_context(tc.tile_pool(name="pnum", bufs=2, space="PSUM"))
                pct_pool = p2s.enter_context(tc.tile_pool(name="pct", bufs=2, space="PSUM"))
                po_pool = p2s.enter_context(tc.tile_pool(name="po", bufs=2, space="PSUM"))

                ctx_q = {}   # st -> ctx tile
                ctxT_q = {}  # st -> ctxT tile

                def stage_num(st):
                    s0 = st * 128
                    pnums = [pnum_pool.tile([128, 2, 130], f32, tag=f"pnum{i}", name=f"pnum{i}") for i in range(2)]
                    for p in range(4):
                        nc.tensor.matmul(pnums[p // 2][:, p % 2, :], ET[:, p, s0:s0 + 128], kvbd[p], start=True, stop=True)
                    ctxs = p2.tile([128, OG], bf16, tag="ctx", name="ctxs")
                    for i in range(2):
                        pn4 = pnums[i].rearrange("p j (two c) -> p (j two) c", two=2)  # [128, 4, 65]
                        r4 = p2.tile([128, 4, 1], f32, tag="r", name="r4")
                        nc.vector.reciprocal(r4, pn4[:, :, 64:65])
                        ctx4 = ctxs[:, i * 256:(i + 1) * 256].rearrange("p (j c) -> p j c", c=64)
                        nc.vector.tensor_mul(ctx4, pn4[:, :, 0:64], r4.to_broadcast([128, 4, 64]))
                    ctx_q[st] = ctxs

                def stage_ctxT(st):
                    ctxs = ctx_q.pop(st)
                    pct = pct_pool.tile([128, 512], bf16, tag="pct", name="pct")
                    for eb in range(4):
                        nc.tensor.transpose(pct[:, eb * 128:(eb + 1) * 128], ctxs[:, eb * 128:(eb + 1) * 128], ident)
                    ctxT = p2.tile([128, 4, 128], bf16, tag="ctxT", name="ctxT")
                    nc.scalar.copy(out=ctxT, in_=pct.rearrange("p (j s) -> p j s", j=4))
                    ctxT_q[st] = ctxT

                def stage_oproj(st):
                    s0 = st * 128
                    ctxT = ctxT_q.pop(st)
                    outsb = p2.tile([128, D], bf16, tag="outsb", name="outsb")
                    for half in range(2):
                        po = po_pool.tile([128, 512], f32, tag="po", name="po")
                        for eb in range(4):
                            nc.tensor.matmul(po, ctxT[:, eb, :],
                                             wo_sb[:, eb, half * 512:(half + 1) * 512],
                                             start=(eb == 0), stop=(eb == 3))
                        if half == 0:
                            nc.scalar.copy(out=outsb[:, 0:512], in_=po)
                        else:
                            nc.vector.tensor_copy(outsb[:, 512:1024], po)
                    # gpsimd queue is idle in phase 2; sync is clogged with sems
                    nc.gpsimd.dma_start(out=out[s0:s0 + 128, :], in_=outsb)


                for st in range(NT):
                    stage_num(st)
                    if st >= 1:
                        stage_ctxT(st - 1)
                    if st >= 2:
                        stage_oproj(st - 2)
                stage_ctxT(NT - 1)
                stage_oproj(NT - 2)
                stage_oproj(NT - 1)

    nc.compile()
    return nc


_LAST_RESULT = None


def kernel(q, k, v, mask, Wq, bq, Wk, bk, Wv, bv, Wo, bo):
    global _LAST_RESULT
    import ml_dtypes
    from concourse.bass_utils import run_bass_kernel_spmd

    q = np.asarray(q, np.float32)
    k = np.asarray(k, np.float32)
    v = np.asarray(v, np.float32)
    mask = np.asarray(mask)
    Wq = np.asarray(Wq, np.float32)
    Wk = np.asarray(Wk, np.float32)
    Wv = np.asarray(Wv, np.float32)
    Wo = np.asarray(Wo, np.float32)
    bq = np.asarray(bq, np.float32)
    bk = np.asarray(bk, np.float32)
    bv = np.asarray(bv, np.float32)
    bo = np.asarray(bo, np.float32)

    nc = _build(bool(np.any(bk) or np.any(bv)))

    bf = ml_dtypes.bfloat16
    f8 = ml_dtypes.float8_e4m3
    in_maps = []
    xts = {}
    def wblocked(wt, t):
        # [T*128, O] -> [128 p, T d-chunks, O] (contiguous per partition)
        return np.ascontiguousarray(wt.reshape(t, 128, -1).transpose(1, 0, 2))

    def blocked(xt):
        # [D, S] -> [128 p, NT s-tiles, 8 d-chunks, 128 s] (contiguous per
        # partition per s-tile range)
        return np.ascontiguousarray(xt.reshape(8, 128, NT, 128).transpose(1, 2, 0, 3))

    for b in range(B):
        xts[b] = (
            np.ascontiguousarray(q[b].T).astype(f8),
            blocked(np.ascontiguousarray(k[b].T).astype(f8)),
            blocked(np.ascontiguousarray(v[b].T).astype(bf)),
        )
    for core in range(NCORES):
        b, g = core // 2, core % 2
        sl = slice(g * OG, (g + 1) * OG)
        maskf = mask[b, 0, 0, :].astype(np.float32).reshape(NT, 128).T.copy()
        xq8, xk8, xv8 = xts[b]
        in_maps.append({
            "xqt": xq8,
            "xkt": xk8,
            "xvt": xv8,
            "wqt": wblocked(np.ascontiguousarray(Wq[sl, :].T).astype(f8), 8),
            "wkt": wblocked(np.ascontiguousarray(Wk[sl, :].T).astype(f8), 8),
            "wvt": wblocked(np.ascontiguousarray(Wv[sl, :].T).astype(bf), 8),
            "wot": wblocked(np.ascontiguousarray(Wo[:, sl].T).astype(bf), 4),
            "bqs": np.ascontiguousarray((bq[sl] * SCALE).reshape(4, 128).T),
            "bk": bk[sl].reshape(1, OG).copy(),
            "bv": bv[sl].reshape(1, OG).copy(),
            "maskf": maskf,
        })

    res = run_bass_kernel_spmd(nc, in_maps, list(range(NCORES)))
    _LAST_RESULT = res

    outp = np.empty((B, S, D), np.float32)
    for b in range(B):
        outp[b] = (res.results[2 * b]["out"].astype(np.float32)
                   + res.results[2 * b + 1]["out"].astype(np.float32)
                   + bo[None, :])
    return outp


# revision 42
# speedup vs baseline: 1.1881x; 1.0065x over previous
"""Trainium2 Bass kernel for nn_MultiHeadAttention_4810363372776 (linear attention).

Sharding: data-parallel over batch (4) x tensor-parallel over head groups (2).
Core i handles batch i//2, heads [8*(i%2), 8*(i%2)+8). Each core computes its
partial output projection; the host sums the two head-group partials per batch.

q/k/v are pre-transposed to [D, S] on the host (fp8 for q/k, bf16 for v), so
the device does no input transposes. q/k projections run as fp8 DoubleRow
matmuls; the v path (v-proj, kv, ctx, out-proj) stays bf16 — any fp8 stage
there costs ~3.6% l2 error on the output, which exceeds the 2e-2 gate.
"""

import functools
import numpy as np

B, S, D, H = 4, 4096, 1024, 16
DK = D // H          # 64
OG = D // 2          # 512 per-core head-group width (8 heads)
NCORES = 8
SCALE = 1.0 / 8.0    # 1/sqrt(DK)
NT = S // 128        # 32 s-tiles
MACRO = 1024
NM = S // MACRO      # 4 macros
NU = MACRO // 128    # 8 s-tiles per macro


@functools.lru_cache(maxsize=2)
def _build(kv_bias=False):
    import concourse.bass as bass  # noqa: F401
    from concourse import bacc
    import concourse.mybir as mybir
    import concourse.tile as tile
    from concourse.masks import make_identity
    from contextlib import ExitStack

    f32 = mybir.dt.float32
    bf16 = mybir.dt.bfloat16
    fp8 = mybir.dt.float8e4
    DR = mybir.MatmulPerfMode.DoubleRow
    EXP = mybir.ActivationFunctionType.Exp
    COPY = mybir.ActivationFunctionType.Copy
    AXX = mybir.AxisListType.X
    ADD = mybir.AluOpType.add

    nc = bacc.Bacc()

    # xk/xv are tile-blocked [p, s-tile, d-chunk, s-in-tile] so each macro
    # load is one contiguous line per partition (128 fat descriptors).
    xqt = nc.declare_dram_parameter("xqt", [D, S], fp8, isOutput=False)
    xkt = nc.declare_dram_parameter("xkt", [128, NT, 8, 128], fp8, isOutput=False)
    xvt = nc.declare_dram_parameter("xvt", [128, NT, 8, 128], bf16, isOutput=False)
    wqt = nc.declare_dram_parameter("wqt", [128, 8, OG], fp8, isOutput=False)
    wkt = nc.declare_dram_parameter("wkt", [128, 8, OG], fp8, isOutput=False)
    wvt = nc.declare_dram_parameter("wvt", [128, 8, OG], bf16, isOutput=False)
    wot = nc.declare_dram_parameter("wot", [128, 4, D], bf16, isOutput=False)
    bqsp = nc.declare_dram_parameter("bqs", [128, 4], f32, isOutput=False)
    bkp = nc.declare_dram_parameter("bk", [1, OG], f32, isOutput=False)
    bvp = nc.declare_dram_parameter("bv", [1, OG], f32, isOutput=False)
    maskp = nc.declare_dram_parameter("maskf", [128, NT], f32, isOutput=False)
    out = nc.declare_dram_parameter("out", [S, D], bf16, isOutput=True)

    with tile.TileContext(nc) as tc:
        with ExitStack() as ctx:
            singles = ctx.enter_context(tc.tile_pool(name="singles", bufs=1))

            ident = singles.tile([128, 128], bf16)
            make_identity(nc, ident)

            # all phase-1 loads go on the gpsimd queue in exact need-order
            # (single-queue FIFO = strict priority at full bandwidth)
            wk_sb = singles.tile([128, 8, OG], fp8, tag="wk")
            mask_sb = singles.tile([128, NT], f32, tag="mask")
            wv_sb = singles.tile([128, 8, OG], bf16, tag="wv")
            bqs_sb = singles.tile([128, 4], f32, tag="bqs")
            wq_sb = singles.tile([128, 8, OG], fp8, tag="wq")
            wo_sb = singles.tile([128, 4, D], bf16, tag="wo")
            nc.gpsimd.dma_start(out=wk_sb, in_=wkt[:, :, :])
            if kv_bias:
                bk_bc = singles.tile([128, OG], f32, tag="bk_bc")
                nc.gpsimd.dma_start(out=bk_bc, in_=bkp[:, :].partition_broadcast(128))
                bv_bc = singles.tile([128, OG], f32, tag="bv_bc")
                nc.gpsimd.dma_start(out=bv_bc, in_=bvp[:, :].partition_broadcast(128))
            # exp(q_hat * scale), stored [o (4 blocks of 128 = head pairs), s]
            ET = singles.tile([128, 4, S], bf16, tag="ET")
            # block-diag [kv | ksum] per head pair; memset early (off critical path)
            kvbd = [singles.tile([128, 130], bf16, tag=f"kvbd{p}", name=f"kvbd{p}") for p in range(4)]
            for p in range(4):
                nc.vector.memset(kvbd[p], 0.0)

            # ---------------- phase 1 ----------------
            with ExitStack() as p1:
                pacc_pool = p1.enter_context(tc.tile_pool(name="pacc", bufs=1, space="PSUM"))
                # two chains per bank; bank-wide has_written clear happens once (st==0, even pair)
                kvps = [pacc_pool.tile([128, 2, 129], f32, tag=f"kvacc{i}", name=f"kvacc{i}") for i in range(2)]
                xin_pool = p1.enter_context(tc.tile_pool(name="xin", bufs=2))
                kvf_pool = p1.enter_context(tc.tile_pool(name="kvf", bufs=3))
                pkv_pool = p1.enter_context(tc.tile_pool(name="pkv", bufs=4, space="PSUM"))

                # PE warmup: get HAM to K=8/8 before the first real matmul
                # lands (accumulation group, result never read)
                pwarm_pool = p1.enter_context(tc.tile_pool(name="pwarm", bufs=1, space="PSUM"))
                wps = pwarm_pool.tile([128, 128], f32, tag="warm")
                for i in range(38):
                    nc.tensor.matmul(wps, ident, ident, start=(i == 0), stop=(i == 37))

                pending = None  # (kf, vf, st) deferred kv accumulation

                def flush_kv(pending):
                    kf, vf, pst = pending
                    for p in range(4):
                        nc.tensor.matmul(
                            kvps[p // 2][:, p % 2, 0:129],
                            kf[:, 2 * p:2 * p + 2, :],
                            vf[:, p, 0:129],
                            start=(pst == 0 and p % 2 == 0),
                            stop=(pst == NT - 1),
                            skip_group_check=True,
                        )

                # Variable-width macros: small ones first so compute starts
                # as soon as the first slices land (deps are tile-granular).
                MACROS = [256, 256, 512, 1024, 1024, 1024]
                assert sum(MACROS) == S
                s_base = 0
                for a, W in enumerate(MACROS):
                    NUw = W // 128
                    st0 = s_base // 128
                    # k first (first compute), q last (needed only at q-chunk)
                    xk_sb = xin_pool.tile([128, 8, 8, 128], fp8, tag="xk")
                    nc.gpsimd.dma_start(out=xk_sb[:, 0:NUw], in_=xkt[:, st0:st0 + NUw])
                    if a == 0:
                        nc.gpsimd.dma_start(out=wv_sb, in_=wvt[:, :, :])
                    xv_sb = xin_pool.tile([128, 8, 8, 128], bf16, tag="xv")
                    nc.gpsimd.dma_start(out=xv_sb[:, 0:NUw], in_=xvt[:, st0:st0 + NUw])
                    if a == 0:
                        nc.gpsimd.dma_start(out=mask_sb, in_=maskp[:, :])
                        nc.gpsimd.dma_start(out=bqs_sb, in_=bqsp[:, :])
                        nc.gpsimd.dma_start(out=wq_sb, in_=wqt[:, :, :])
                    xq_sb = xin_pool.tile([128, 8, MACRO], fp8, tag="xq")
                    nc.gpsimd.dma_start(out=xq_sb[:, :, 0:W], in_=xqt[:, s_base:s_base + W].rearrange("(t p) s -> p t s", p=128))

                    for u in range(NUw):
                        st = st0 + u
                        sl = slice(u * 128, (u + 1) * 128)

                        # k projection (fp8 DoubleRow over 1024 contract)
                        pk = pkv_pool.tile([128, OG], f32, tag="pkv")
                        for t2 in range(4):
                            nc.tensor.matmul(pk, xk_sb[:, u, 2 * t2:2 * t2 + 2, :],
                                             wk_sb[:, 2 * t2:2 * t2 + 2, :],
                                             start=(t2 == 0), stop=(t2 == 3), perf_mode=DR)
                        if kv_bias:
                            nc.vector.tensor_add(pk, pk, bk_bc)
                        ek = kvf_pool.tile([128, OG], bf16, tag="ek")
                        nc.scalar.activation(ek, pk, EXP, scale=SCALE)
                        rows = kvf_pool.tile([128, 8], f32, tag="rows")
                        nc.vector.tensor_reduce(rows, ek.rearrange("p (h e) -> p h e", h=8), axis=AXX, op=ADD)
                        nc.vector.reciprocal(rows, rows)
                        nc.vector.tensor_scalar_mul(rows, rows, mask_sb[:, st:st + 1])
                        kf = kvf_pool.tile([128, 8, DK], bf16, tag="kf")
                        nc.vector.tensor_mul(
                            kf,
                            ek.rearrange("p (h e) -> p h e", h=8),
                            rows[:, :, None].to_broadcast([128, 8, DK]),
                        )

                        # v projection (bf16)
                        pv = pkv_pool.tile([128, OG], f32, tag="pkv")
                        for t in range(8):
                            nc.tensor.matmul(pv, xv_sb[:, u, t, :], wv_sb[:, t, :],
                                             start=(t == 0), stop=(t == 7))
                        if kv_bias:
                            nc.vector.tensor_add(pv, pv, bv_bc)
                        vf = kvf_pool.tile([128, 4, 130], bf16, tag="vf")
                        nc.scalar.activation(vf[:, :, 0:128], pv.rearrange("p (j s) -> p j s", j=4), COPY, scale=mask_sb[:, st:st + 1])
                        nc.vector.memset(vf[:, :, 128:129], 1.0)

                        # deferred kv accumulation for the previous s-tile
                        if pending is not None:
                            flush_kv(pending)
                        pending = (kf, vf, st)

                        # q projection per chunk (<=512 wide), output [o, s]
                        QW = min(512, W)
                        if (u + 1) * 128 % QW == 0:
                            c = (u * 128) // QW
                            soff = s_base + c * QW
                            last_chunk = (a == len(MACROS) - 1) and (u == NUw - 1)
                            for ob in range(4):
                                pq = pkv_pool.tile([128, 512], f32, tag="pkv")
                                for t2 in range(4):
                                    nc.tensor.matmul(pq[:, 0:QW], wq_sb[:, 2 * t2:2 * t2 + 2, ob * 128:(ob + 1) * 128],
                                                     xq_sb[:, 2 * t2:2 * t2 + 2, c * QW:(c + 1) * QW],
                                                     start=(t2 == 0), stop=(t2 == 3), perf_mode=DR)
                                nc.scalar.activation(ET[:, ob, soff:soff + QW], pq[:, 0:QW], EXP, bias=bqs_sb[:, ob:ob + 1], scale=SCALE)
                                if last_chunk and ob == 0:
                                    # flush the final tile's kv now so the
                                    # kvbd build overlaps the rest of q-proj
                                    flush_kv(pending)
                                    pending = None

                    if a == 0:
                        # wo is phase-2-only; load it once the urgent head
                        # transfers are queued
                        nc.sync.dma_start(out=wo_sb, in_=wot[:, :, :])
                    s_base += W

                if pending is not None:
                    flush_kv(pending)

                # build block-diag [kv | ksum] tiles (bf16) on DVE only —
                # the ACT queue is still draining the final ET evacuations
                for p in range(4):
                    ps = kvps[p // 2][:, p % 2]
                    nc.vector.tensor_copy(kvbd[p][0:64, 0:64], ps[0:64, 0:64])
                    nc.vector.tensor_copy(kvbd[p][0:64, 64:65], ps[0:64, 128:129])
                    nc.vector.tensor_copy(kvbd[p][64:128, 65:129], ps[64:128, 64:128])
                    nc.vector.tensor_copy(kvbd[p][64:128, 129:130], ps[64:128, 128:129])

            # ---------------- phase 2 ----------------
            # stages per s-tile: num -> (DVE) ctx -> (PE) ctxT -> (ACT) evac -> (PE) out-proj
            # software-pipelined: ctxT lags one tile, out-proj lags two.
            with ExitStack() as p2s:
                p2 = p2s.enter_context(tc.tile_pool(name="p2", bufs=3))
                pnum_pool = p2s.enter_context(tc.tile_pool(name="pnum", bufs=2, space="PSUM"))
                pct_pool = p2s.enter_context(tc.tile_pool(name="pct", bufs=2, space="PSUM"))
                po_pool = p2s.enter_context(tc.tile_pool(name="po", bufs=2, space="PSUM"))

                ctx_q = {}   # st -> ctx tile
                ctxT_q = {}  # st -> ctxT tile

                def stage_num(st):
                    s0 = st * 128
                    pnums = [pnum_pool.tile([128, 2, 130], f32, tag=f"pnum{i}", name=f"pnum{i}") for i in range(2)]
                    for p in range(4):
                        nc.tensor.matmul(pnums[p // 2][:, p % 2, :], ET[:, p, s0:s0 + 128], kvbd[p], start=True, stop=True)
                    ctxs = p2.tile([128, OG], bf16, tag="ctx", name="ctxs")
                    for i in range(2):
                        pn4 = pnums[i].rearrange("p j (two c) -> p (j two) c", two=2)  # [128, 4, 65]
                        r4 = p2.tile([128, 4, 1], f32, tag="r", name="r4")
                        nc.vector.reciprocal(r4, pn4[:, :, 64:65])
                        ctx4 = ctxs[:, i * 256:(i + 1) * 256].rearrange("p (j c) -> p j c", c=64)
                        nc.vector.tensor_mul(ctx4, pn4[:, :, 0:64], r4.to_broadcast([128, 4, 64]))
                    ctx_q[st] = ctxs

                def stage_ctxT(st):
                    ctxs = ctx_q.pop(st)
                    pct = pct_pool.tile([128, 512], bf16, tag="pct", name="pct")
                    for eb in range(4):
                        nc.tensor.transpose(pct[:, eb * 128:(eb + 1) * 128], ctxs[:, eb * 128:(eb + 1) * 128], ident)
                    ctxT = p2.tile([128, 4, 128], bf16, tag="ctxT", name="ctxT")
                    nc.scalar.copy(out=ctxT, in_=pct.rearrange("p (j s) -> p j s", j=4))
                    ctxT_q[st] = ctxT

                def stage_oproj(st):
                    s0 = st * 128
                    ctxT = ctxT_q.pop(st)
                    outsb = p2.tile([128, D], bf16, tag="outsb", name="outsb")
                    for half in range(2):
                        po = po_pool.tile([128, 512], f32, tag="po", name="po")
                        for eb in range(4):
                            nc.tensor.matmul(po, ctxT[:, eb, :],
                                             wo_sb[:, eb, half * 512:(half + 1) * 512],
                                             start=(eb == 0), stop=(eb == 3))
                        if half == 0:
                            nc.scalar.copy(out=outsb[:, 0:512], in_=po)
                        else:
                            nc.vector.tensor_copy(outsb[:, 512:1024], po)
                    # gpsimd queue is idle in phase 2; sync is clogged with sems
                    nc.gpsimd.dma_start(out=out[s0:s0 + 128, :], in_=outsb)


                for st in range(NT):
                    stage_num(st)
                    if st >= 1:
                        stage_ctxT(st - 1)
                    if st >= 2:
                        stage_oproj(st - 2)
                stage_ctxT(NT - 1)
                stage_oproj(NT - 2)
                stage_oproj(NT - 1)

    nc.compile()
    return nc


_LAST_RESULT = None


def kernel(q, k, v, mask, Wq, bq, Wk, bk, Wv, bv, Wo, bo):
    global _LAST_RESULT
    import ml_dtypes
    from concourse.bass_utils import run_bass_kernel_spmd

    q = np.asarray(q, np.float32)
    k = np.asarray(k, np.float32)
    v = np.asarray(v, np.float32)
    mask = np.asarray(mask)
    Wq = np.asarray(Wq, np.float32)
    Wk = np.asarray(Wk, np.float32)
    Wv = np.asarray(Wv, np.float32)
    Wo = np.asarray(Wo, np.float32)
    bq = np.asarray(bq, np.float32)
    bk = np.asarray(bk, np.float32)
    bv = np.asarray(bv, np.float32)
    bo = np.asarray(bo, np.float32)

    nc = _build(bool(np.any(bk) or np.any(bv)))

    bf = ml_dtypes.bfloat16
    f8 = ml_dtypes.float8_e4m3
    in_maps = []
    xts = {}
    def wblocked(wt, t):
        # [T*128, O] -> [128 p, T d-chunks, O] (contiguous per partition)
        return np.ascontiguousarray(wt.reshape(t, 128, -1).transpose(1, 0, 2))

    def blocked(xt):
        # [D, S] -> [128 p, NT s-tiles, 8 d-chunks, 128 s] (contiguous per
        # partition per s-tile range)
        return np.ascontiguousarray(xt.reshape(8, 128, NT, 128).transpose(1, 2, 0, 3))

    for b in range(B):
        xts[b] = (
            np.ascontiguousarray(q[b].T).astype(f8),
            blocked(np.ascontiguousarray(k[b].T).astype(f8)),
            blocked(np.ascontiguousarray(v[b].T).astype(bf)),
        )
    for core in range(NCORES):
        b, g = core // 2, core % 2
        sl = slice(g * OG, (g + 1) * OG)
        maskf = mask[b, 0, 0, :].astype(np.float32).reshape(NT, 128).T.copy()
        xq8, xk8, xv8 = xts[b]
        in_maps.append({
            "xqt": xq8,
            "xkt": xk8,
            "xvt": xv8,
            "wqt": wblocked(np.ascontiguousarray(Wq[sl, :].T).astype(f8), 8),
            "wkt": wblocked(np.ascontiguousarray(Wk[sl, :].T).astype(f8), 8),
            "wvt": wblocked(np.ascontiguousarray(Wv[sl, :].T).astype(bf), 8),
            "wot": wblocked(np.ascontiguousarray(Wo[:, sl].T).astype(bf), 4),
            "bqs": np.ascontiguousarray((bq[sl] * SCALE).reshape(4, 128).T),
            "bk": bk[sl].reshape(1, OG).copy(),
            "bv": bv[sl].reshape(1, OG).copy(),
            "maskf": maskf,
        })

    res = run_bass_kernel_spmd(nc, in_maps, list(range(NCORES)))
    _LAST_RESULT = res

    outp = np.empty((B, S, D), np.float32)
    for b in range(B):
        outp[b] = (res.results[2 * b]["out"].astype(np.float32)
                   + res.results[2 * b + 1]["out"].astype(np.float32)
                   + bo[None, :])
    return outp
